# revision 3
# baseline (speedup 1.0000x reference)
"""CausalLocalAttention Trainium2 kernel (8-core SPMD, data-parallel).

Problem: B=4, S=4096, D=1024, H=16 heads, DH=64, window W=256 (block-local
causal attention), fp32 in/out.

Sharding: the 16384 tokens split into 8 contiguous 2048-token shards (block-
and batch-aligned), one per NeuronCore. Weights replicated. No collectives.

Per-core layout strategy (all matmuls fp32r, contraction on partitions):
  xT [din, t] (host-transposed)  -> QT/KT = Wq/Wk as lhsT (feature-major)
                                 -> V    = xT as lhsT    (token-major)
  S^T[k,q]  = KT-half.T @ QT  per (head, block); causal mask added on PSUM
  P^T       = exp(S^T * scale)     (ACT, fp32r)
  attn^T,l  = [V|1].T @ P^T        (M=65: row 64 = softmax denominator l)
  attn_norm = attn^T * bcast(1/l)  (PE K=1 bcast + DVE recip/mul)
  y         = attn^T as lhsT @ Wo  (token-major) -> DMA out
"""
import sys
sys.path.insert(0, "/opt/trn_rl_repo")
import os
import numpy as np
from contextlib import ExitStack

import concourse.bass as bass
import concourse.tile as tile
from concourse import mybir
from concourse.bass_utils import run_bass_kernel_spmd
from concourse.vector_clock import ScopedClock

F32 = mybir.dt.float32
F32R = mybir.dt.float32r

N_CORES = 8
B, S, D = 4, 4096, 1024
H, W, DH = 16, 256, 64
T_CORE = (B * S) // N_CORES      # 2048 tokens per core
CHUNK = 1024                     # tokens processed per chunk
N_CHUNK = T_CORE // CHUNK        # 2
SCALE = 1.0 / float(np.sqrt(DH))
NEG = -1e30


# ---------------------------------------------------------------------------
# walrus on this toolchain allows only one sem wait per instruction; split
# extras onto same-engine NoOps inserted right before the instruction.
def _split_sync_waits(nc: bass.Bass, max_waits: int = 1) -> None:
    for fn in nc.m.functions:
        for bb in fn.blocks:
            insts = bb.instructions
            if not any(
                i.sync_info and i.sync_info.on_wait
                and len(i.sync_info.on_wait) > max_waits
                for i in insts
            ):
                continue
            new = []
            for inst in insts:
                si = inst.sync_info
                waits = list(si.on_wait) if (si and si.on_wait) else []
                if len(waits) > max_waits:
                    n_excess = len(waits) - max_waits
                    for w in waits[:n_excess]:
                        nop = mybir.InstNoOp(
                            name=f"WSPLIT-{nc.next_id()}", ins=[], outs=[]
                        )
                        nop.engine = inst.engine
                        nop.sync_info = mybir.SyncInfo(on_wait=[w], on_update=[])
                        nc.register_instruction(nop)
                        new.append(nop)
                    si.on_wait = waits[n_excess:]
                new.append(inst)
            bb.instructions = new


class _WTileContext(tile.TileContext):
    def _drain_and_barrier(self, tick_clock, wait_clock):
        drain_inst = self.nc.sync.drain()
        wait_clock.add_sem_waits(
            drain_inst.ins, ScopedClock({None: tick_clock.global_clock})
        )
        self.nc.all_engine_barrier()
        assert self.sems is not None
        popped = self.nc._tile_sem_poison_stack.pop()
        assert popped is self._sem_poison
        self.nc.clear_and_free_semaphores(list(self.sems.allocated().values()))
        self.nc.all_engine_barrier()

    def __exit__(self, exc_type, exc_val, exc_tb):
        ret = super().__exit__(exc_type, exc_val, exc_tb)
        if exc_type is None:
            _split_sync_waits(self.nc)
        return ret


# ---------------------------------------------------------------------------
def build_program(repeat: int = 1) -> bass.Bass:
    nc = bass.Bass("TRN2", target_bir_lowering=False, debug=False,
                   num_devices=N_CORES)

    xt_ap = nc.dram_tensor("xt", [D, T_CORE], F32R, kind="ExternalInput").ap()
    wq_ap = nc.dram_tensor("wq", [D, D], F32R, kind="ExternalInput").ap()
    wk_ap = nc.dram_tensor("wk", [D, D], F32R, kind="ExternalInput").ap()
    wv_ap = nc.dram_tensor("wv", [D, D], F32R, kind="ExternalInput").ap()
    wo_ap = nc.dram_tensor("wo", [D, D], F32R, kind="ExternalInput").ap()
    bq_ap = nc.dram_tensor("bqr", [128, 8], F32, kind="ExternalInput").ap()
    bk_ap = nc.dram_tensor("bkr", [128, 8], F32, kind="ExternalInput").ap()
    bv_ap = nc.dram_tensor("bvb", [128, D], F32, kind="ExternalInput").ap()
    bo_ap = nc.dram_tensor("bob", [128, D], F32, kind="ExternalInput").ap()
    m1_ap = nc.dram_tensor("m1", [128, W], F32, kind="ExternalInput").ap()
    m2_ap = nc.dram_tensor("m2", [128, W], F32, kind="ExternalInput").ap()
    onesb_ap = nc.dram_tensor("onesb", [1, 64], F32R, kind="ExternalInput").ap()
    y_ap = nc.dram_tensor("y", [T_CORE, D], F32, kind="ExternalOutput").ap()

    KT_N = CHUNK // 512          # moving-dim chunks per projection (2)
    NT = CHUNK // 128            # t-tiles per chunk (8)
    NB = CHUNK // W              # blocks per chunk (4)

    with _WTileContext(nc) as tc, ExitStack() as top:
        consts = top.enter_context(tc.tile_pool(name="consts", bufs=1))
        m1_sb = consts.tile([128, W], F32)
        m2_sb = consts.tile([128, W], F32)
        bq_sb = consts.tile([128, 8], F32)
        bk_sb = consts.tile([128, 8], F32)
        bv_sb = consts.tile([128, D], F32)
        bo_sb = consts.tile([128, D], F32)
        onesb_sb = consts.tile([1, 64], F32R)
        nc.sync.dma_start(m1_sb[:], m1_ap[:])
        nc.sync.dma_start(m2_sb[:], m2_ap[:])
        nc.sync.dma_start(bq_sb[:], bq_ap[:])
        nc.sync.dma_start(bk_sb[:], bk_ap[:])
        nc.sync.dma_start(bv_sb[:], bv_ap[:])
        nc.sync.dma_start(bo_sb[:], bo_ap[:])
        nc.sync.dma_start(onesb_sb[:], onesb_ap[:])
        bv_h = bv_sb[:].rearrange("p (h d) -> p h d", h=H)

        for _rep in range(repeat):
            for c in range(N_CHUNK):
                with ExitStack() as chunk_ctx:
                    p_qkv = chunk_ctx.enter_context(
                        tc.tile_pool(name="p_qkv", bufs=1)
                    )
                    qt_sb = p_qkv.tile([128, 8, CHUNK], F32R, tag="qt")
                    kt_sb = p_qkv.tile([128, 8, CHUNK], F32R, tag="kt")
                    v3_sb = p_qkv.tile([128, NT, H, DH + 1], F32R, tag="v3")

                    with ExitStack() as x_ctx:
                        p_x = x_ctx.enter_context(
                            tc.tile_pool(name="p_x", bufs=1)
                        )
                        xt_sb = p_x.tile([128, 8, CHUNK], F32R, tag="xt")
                        nc.sync.dma_start(
                            xt_sb[:],
                            xt_ap[:, c * CHUNK:(c + 1) * CHUNK].rearrange(
                                "(k p) t -> p k t", p=128
                            ),
                        )

                        # ---- Q / K projections (feature-major) ----
                        for w_ap, bias_sb, dst in (
                            (wq_ap, bq_sb, qt_sb),
                            (wk_ap, bk_sb, kt_sb),
                        ):
                            with ExitStack() as ph:
                                p_w = ph.enter_context(
                                    tc.tile_pool(name="p_w", bufs=2)
                                )
                                ps = ph.enter_context(
                                    tc.tile_pool(name="ps", bufs=4, space="PSUM")
                                )
                                for m in range(8):
                                    w_buf = p_w.tile([128, 8, 128], F32R, tag="w")
                                    nc.sync.dma_start(
                                        w_buf[:],
                                        w_ap[:, m * 128:(m + 1) * 128].rearrange(
                                            "(k p) c -> p k c", p=128
                                        ),
                                    )
                                    for n in range(KT_N):
                                        acc = ps.tile([128, 512], F32, tag="acc")
                                        for k in range(8):
                                            nc.tensor.matmul(
                                                acc[:],
                                                w_buf[:, k, :],
                                                xt_sb[:, k, n * 512:(n + 1) * 512],
                                                start=(k == 0), stop=(k == 7),
                                            )
                                        nc.scalar.activation(
                                            dst[:, m, n * 512:(n + 1) * 512],
                                            acc[:],
                                            mybir.ActivationFunctionType.Identity,
                                            bias=bias_sb[:, m:m + 1],
                                        )

                        # ---- V projection (token-major, with ones column) ----
                        with ExitStack() as ph:
                            p_w = ph.enter_context(
                                tc.tile_pool(name="p_wv", bufs=2)
                            )
                            ps = ph.enter_context(
                                tc.tile_pool(name="ps", bufs=4, space="PSUM")
                            )
                            for h2 in range(2):
                                w_buf = p_w.tile([128, 8, 512], F32R, tag="wv")
                                nc.sync.dma_start(
                                    w_buf[:],
                                    wv_ap[:, h2 * 512:(h2 + 1) * 512].rearrange(
                                        "(k p) c -> p k c", p=128
                                    ),
                                )
                                for i in range(NT):
                                    acc = ps.tile([128, 512], F32, tag="acc")
                                    for k in range(8):
                                        nc.tensor.matmul(
                                            acc[:],
                                            xt_sb[:, k, i * 128:(i + 1) * 128],
                                            w_buf[:, k, :],
                                            start=(k == 0), stop=(k == 7),
                                        )
                                    nc.vector.tensor_add(
                                        v3_sb[:, i, h2 * 8:(h2 + 1) * 8, 0:DH],
                                        acc[:].rearrange("p (h d) -> p h d", h=8),
                                        bv_h[:, h2 * 8:(h2 + 1) * 8, :],
                                    )
                            for i in range(NT):
                                nc.gpsimd.memset(v3_sb[:, i, :, DH:DH + 1].bitcast(F32), 1.0)

                    # ---- attention + output projection ----
                    with ExitStack() as attn_ctx:
                        p_at = attn_ctx.enter_context(
                            tc.tile_pool(name="p_at", bufs=1)
                        )
                        at_sb = p_at.tile([128, 8, CHUNK], F32R, tag="at")

                        blocks_ctx = ExitStack()
                        p_work = blocks_ctx.enter_context(
                            tc.tile_pool(name="p_work", bufs=3)
                        )
                        ps_st = blocks_ctx.enter_context(
                            tc.tile_pool(name="ps_st", bufs=2, space="PSUM")
                        )
                        ps_pv = blocks_ctx.enter_context(
                            tc.tile_pool(name="ps_pv", bufs=3, space="PSUM")
                        )
                        ps_lb = blocks_ctx.enter_context(
                            tc.tile_pool(name="ps_lb", bufs=2, space="PSUM")
                        )
                        for b in range(NB):
                            t0 = b * W
                            for h in range(H):
                                hp = (h % 2) * 64
                                j = h // 2
                                qh = qt_sb[hp:hp + 64, j, t0:t0 + W]
                                kh = kt_sb[hp:hp + 64, j, t0:t0 + W]
                                st = ps_st.tile([128, 2 * W], F32, tag="st")
                                nc.tensor.matmul(
                                    st[:, 0:W], kh[:, 0:128], qh[:],
                                    start=True, stop=True,
                                    tile_position=(hp, 0),
                                )
                                nc.tensor.matmul(
                                    st[:, W:2 * W], kh[:, 128:256], qh[:],
                                    start=True, stop=True,
                                    tile_position=(hp, 0),
                                )
                                nc.vector.tensor_add(
                                    st[:, 0:W], st[:, 0:W], m1_sb[:]
                                )
                                nc.vector.tensor_add(
                                    st[:, W:2 * W], st[:, W:2 * W], m2_sb[:]
                                )
                                pt = p_work.tile([128, 2 * W], F32R, tag="pt")
                                nc.scalar.activation(
                                    pt[:], st[:],
                                    mybir.ActivationFunctionType.Exp,
                                    scale=SCALE,
                                )
                                pv = ps_pv.tile([DH + 1, W], F32, tag="pv")
                                for ik in range(2):
                                    nc.tensor.matmul(
                                        pv[:],
                                        v3_sb[:, 2 * b + ik, h, :],
                                        pt[:, ik * W:(ik + 1) * W],
                                        start=(ik == 0), stop=(ik == 1),
                                    )
                                l_sb = p_work.tile([1, W], F32R, tag="l")
                                nc.scalar.copy(l_sb[:], pv[DH:DH + 1, :])
                                lb = ps_lb.tile([64, W], F32, tag="lb")
                                nc.tensor.matmul(
                                    lb[:], onesb_sb[:], l_sb[:],
                                    start=True, stop=True,
                                )
                                rec = p_work.tile([64, W], F32, tag="rec")
                                nc.vector.reciprocal(rec[:], lb[:])
                                nc.vector.tensor_mul(
                                    at_sb[hp:hp + 64, j, t0:t0 + W],
                                    pv[0:DH, :], rec[:],
                                )

                        blocks_ctx.close()

                        # ---- output projection ----
                        with ExitStack() as ph:
                            p_w = ph.enter_context(
                                tc.tile_pool(name="p_wo", bufs=2)
                            )
                            p_y = ph.enter_context(
                                tc.tile_pool(name="p_y", bufs=2)
                            )
                            ps = ph.enter_context(
                                tc.tile_pool(name="ps", bufs=4, space="PSUM")
                            )
                            for h2 in range(2):
                                w_buf = p_w.tile([128, 8, 512], F32R, tag="wo")
                                nc.sync.dma_start(
                                    w_buf[:],
                                    wo_ap[:, h2 * 512:(h2 + 1) * 512].rearrange(
                                        "(k p) c -> p k c", p=128
                                    ),
                                )
                                for i in range(NT):
                                    acc = ps.tile([128, 512], F32, tag="acc")
                                    for k in range(8):
                                        nc.tensor.matmul(
                                            acc[:],
                                            at_sb[:, k, i * 128:(i + 1) * 128],
                                            w_buf[:, k, :],
                                            start=(k == 0), stop=(k == 7),
                                        )
                                    y_t = p_y.tile([128, 512], F32, tag="y")
                                    nc.vector.tensor_add(
                                        y_t[:], acc[:],
                                        bo_sb[:, h2 * 512:(h2 + 1) * 512],
                                    )
                                    nc.sync.dma_start(
                                        y_ap[c * CHUNK + i * 128:
                                             c * CHUNK + (i + 1) * 128,
                                             h2 * 512:(h2 + 1) * 512],
                                        y_t[:],
                                    )
    return nc


# ---------------------------------------------------------------------------
_CACHE: dict = {}


def _host_prep(x, Wq, bq, Wk, bk, Wv, bv, Wo, bo):
    x_flat = np.ascontiguousarray(x.reshape(B * S, D).astype(np.float32))
    m1 = np.zeros((128, W), np.float32)
    m2 = np.zeros((128, W), np.float32)
    for p in range(128):
        m1[p, :p] = NEG
        m2[p, :128 + p] = NEG
    common = {
        "wq": np.ascontiguousarray(Wq, np.float32),
        "wk": np.ascontiguousarray(Wk, np.float32),
        "wv": np.ascontiguousarray(Wv, np.float32),
        "wo": np.ascontiguousarray(Wo, np.float32),
        "bqr": np.ascontiguousarray(np.asarray(bq, np.float32).reshape(8, 128).T),
        "bkr": np.ascontiguousarray(np.asarray(bk, np.float32).reshape(8, 128).T),
        "bvb": np.ascontiguousarray(np.tile(np.asarray(bv, np.float32), (128, 1))),
        "bob": np.ascontiguousarray(np.tile(np.asarray(bo, np.float32), (128, 1))),
        "m1": m1,
        "m2": m2,
        "onesb": np.ones((1, 64), np.float32),
    }
    in_maps = []
    for cix in range(N_CORES):
        xt = np.ascontiguousarray(
            x_flat[cix * T_CORE:(cix + 1) * T_CORE].T
        )
        in_maps.append({"xt": xt, **common})
    return in_maps


def run_on_cores(inputs: dict, repeat: int = 1):
    """Run the SPMD program; returns per-core results list."""
    key = ("nc", repeat)
    if key not in _CACHE:
        _CACHE[key] = build_program(repeat)
    nc = _CACHE[key]
    in_maps = _host_prep(**inputs)
    return run_bass_kernel_spmd(nc, in_maps, list(range(N_CORES)))


def kernel(**inputs) -> np.ndarray:
    res = run_on_cores(inputs, repeat=1)
    y = np.concatenate(
        [res.results[cix]["y"] for cix in range(N_CORES)], axis=0
    )
    return y.reshape(B, S, D).astype(np.float32)
